# revision 41
# baseline (speedup 1.0000x reference)
"""Bass/Trainium2 kernel for batched kNN-interpolate + MSE (nn_KnnMSE).

Reference computation:
  d2[i,j] = ||c2_i - c1_j||^2, masked to same-graph pairs (b1/b2 sorted),
  top-k=8 smallest per target row, w = 1/clip(d2, 1e-16),
  interp = sum(w * f1[idx]) / sum(w),  out = mean((interp - f2)^2).

v3 strategy:
  * b1/b2 sorted => block-diagonal over 64 graphs; one graph per core per
    "slot" (8 slots x 8 cores).  Slot shapes (source width W_s, source
    128-blocks SCH_s, target chunks TCH_s) specialized to the ACTUAL graph
    sizes at call time (graphs sorted by size, dealt into slots of 8; all
    cores run one shared program).
  * All inputs DMA'd into SBUF up front (~10 large DMAs, ~2MB resident).
  * Distance matmul folds in BOTH norm terms (KMM=13 fp16 hi/lo rows):
    PSUM holds d2 directly; no relu needed (d2 >= 2.7e-4 > 0 always).
  * bf16 selection pipeline at DVE 2x rate: negd2 = -d2 (one ACT scaled
    copy, PSUM->SBUF bf16); top8 = max8(negd2); the top-8 weight matrix
    comes out of ONE fused DVE op: W = (negd2 >= thresh) / negd2 (is_ge +
    divide) = -(1/d2) at the 8 nearest, 0 elsewhere, with sumw as the free
    accumulator output.  The minus signs cancel in interp = numer*rsw.
    This removes the full-width f32 reciprocal entirely.
  * W^T via the DMA xbar transpose (16x128 tiles, bf16) on the ~90% idle
    DMA engines - no PE transpose matmuls, no ACT PSUM->SBUF copy.
  * interp = ACT scaled copy of the numer PSUM (scale = mask/sumw rolled
    into one per-row scalar); err = pool subtract (bf16); sum of squares
    via one DVE fused multiply-accumulate per chunk.

Per 128-target chunk:
  PE   : psum = d2   (13-row fp16 hi/lo matmul, W_s wide)
  ACT  : nd = -d2    (scaled copy, bf16)
  DVE  : top8 = max8(nd); W,sumw = (nd >= top8[:,7]) / nd  [one fused op]
  DVE  : rsw = 1/sumw; rswm = rsw * mask
  DMA  : wts = W^T   (xbar transpose, SCH_s x [16,128] tiles)
  PE   : numer = wts^T @ f1  (bf16, PSUM-accum over SCH_s blocks)
  ACT  : interp = numer * rswm  (scaled copy, bf16)
  Pool : err = interp - f2      (bf16; f2/interp are 0 on padded rows)
  DVE  : acc[:, ci] += sum_d err*err  [fused mult-mult + accum]
  finally: per-partition totals -> DRAM; host sums 8x128 values / (N*D).

Self-contained: hardcodes problem constants; slot shapes come from the
actual b1/b2 at call time (host-side prep; program cached per shape set).
"""

import os as _os

import numpy as np

# Problem constants
N = 16384
D = 128
B = 64
KNN = 8
NCORES = 8
NSLOTS = B // NCORES     # 8 slots, one graph per core each
WBUF = 320               # SBUF buffer width for sources (max n1 <= 292)
TMAX = 3                 # max 128-row target chunks per graph
KMM = 13                 # dist contraction: 3x3 hi/lo cross + 2 n1 + 2 n2 rows
BIGC = 100.0             # padded-source coordinate => d2 ~ 3e4 >> real d2

PIPE = _os.environ.get("KNN_PIPE", "1") == "1"
RECIP = _os.environ.get("KNN_RECIP", "act")   # act | f32


def _act_reciprocal(nc, mybir, out, in_, accum_out=None):
    """ACT-engine table reciprocal (bass blocks AF.Reciprocal behind a
    precision ValueError; table accuracy ~1e-3 rel is plenty here)."""
    eng = nc.scalar
    AF = mybir.ActivationFunctionType
    ins = [
        eng.lower_ap(in_),
        mybir.ImmediateValue(dtype=mybir.dt.float32, value=0.0),  # bias
        mybir.ImmediateValue(dtype=mybir.dt.float32, value=1.0),  # scale
        mybir.ImmediateValue(dtype=mybir.dt.float32, value=0.0),  # alpha
    ]
    outs = [eng.lower_ap(out)]
    if accum_out is not None:
        outs.append(eng.lower_ap(accum_out))
    return eng.add_instruction(
        mybir.InstActivation(
            name=nc.get_next_instruction_name(),
            func=AF.Reciprocal,
            ins=ins,
            outs=outs,
        )
    )


def _build_nc(slot_shapes):
    import concourse.bacc as bacc
    import concourse.mybir as mybir
    import concourse.tile as tile

    f32 = mybir.dt.float32
    f16 = mybir.dt.float16
    bf16 = mybir.dt.bfloat16
    OP = mybir.AluOpType
    AF = mybir.ActivationFunctionType

    SCHs = [sh[1] for sh in slot_shapes]
    TCHs = [sh[2] for sh in slot_shapes]
    fo = np.concatenate([[0], np.cumsum(SCHs)])   # f1a block offsets
    to = np.concatenate([[0], np.cumsum(TCHs)])   # chunk offsets
    FTOT = int(fo[-1])
    CTOT = int(to[-1])

    nc = bacc.Bacc("TRN2", target_bir_lowering=False, debug=False)

    c1r_d = nc.dram_tensor("c1r", [KMM, NSLOTS, WBUF], f16, kind="ExternalInput")
    c2t_d = nc.dram_tensor("c2t", [KMM, NSLOTS, TMAX, 128], f16, kind="ExternalInput")
    f1a_d = nc.dram_tensor("f1a", [128, FTOT, D], bf16, kind="ExternalInput")
    f2_d = nc.dram_tensor("f2", [128, CTOT, D], bf16, kind="ExternalInput")
    msk_d = nc.dram_tensor("msk", [128, CTOT], f32, kind="ExternalInput")
    out_d = nc.dram_tensor("out_sums", [128, 1], f32, kind="ExternalOutput")

    LOOK = int(_os.environ.get("KNN_LOOK", "4")) if PIPE else 0
    NW = 3 * (LOOK + 1) + 3
    with tile.TileContext(nc) as tc:
        with (
            tc.tile_pool(name="constp", bufs=1) as constp,
            tc.tile_pool(name="ndp", bufs=8) as ndp,
            tc.tile_pool(name="wp", bufs=NW) as wp,
            tc.tile_pool(name="wtsp", bufs=NW) as wtsp,
            tc.tile_pool(name="errp", bufs=6) as errp,
            tc.tile_pool(name="small", bufs=2 * (LOOK + 2)) as small,
            tc.tile_pool(name="pdp", bufs=5, space="PSUM") as pdp,
            tc.tile_pool(name="pip", bufs=3, space="PSUM") as pip_,
        ):
            acc = constp.tile([128, NSLOTS], f32)
            nc.vector.memset(acc, 0.0)

            # Resident inputs, loaded once up front.
            c1r_t = constp.tile([KMM, NSLOTS, WBUF], f16)
            c2t_t = constp.tile([KMM, NSLOTS, TMAX, 128], f16)
            f1a_t = constp.tile([128, FTOT, D], bf16)
            f2_t = constp.tile([128, CTOT, D], bf16)
            msk_t = constp.tile([128, CTOT], f32)

            # Slot-0 coordinate slices first (tiny) so the first distance
            # matmul can start as early as possible; bulk loads follow.
            W0 = slot_shapes[0][0]
            nc.sync.dma_start(c1r_t[:, 0, :W0], c1r_d[:, 0, :W0])
            nc.sync.dma_start(c2t_t[:, 0], c2t_d[:, 0])
            nc.sync.dma_start(c1r_t[:, 1:], c1r_d[:, 1:])
            nc.gpsimd.dma_start(c2t_t[:, 1:], c2t_d[:, 1:])
            nc.gpsimd.dma_start(msk_t, msk_d[:, :])
            # f1a/f2 are large; split across DMA queues, slot-bounds aligned.
            h1 = int(fo[NSLOTS // 2])
            nc.sync.dma_start(f1a_t[:, :h1], f1a_d[:, :h1])
            nc.sync.dma_start(f1a_t[:, h1:], f1a_d[:, h1:])
            qs = [int(to[i]) for i in (0, 2, 4, 6)] + [CTOT]
            for a, b in zip(qs[:-1], qs[1:]):
                nc.gpsimd.dma_start(f2_t[:, a:b], f2_d[:, a:b])

            state = {}

            def emit_front(s):
                W, SCH, TCH = slot_shapes[s]
                pds, recs, tops, rsws = [], [], [], []
                for t in range(TCH):
                    pd = pdp.tile([128, WBUF], f32, tag="pd")
                    nc.tensor.matmul(
                        pd[:, :W], c2t_t[:, s, t], c1r_t[:, s, :W],
                        start=True, stop=True,
                    )
                    pds.append(pd)
                if RECIP == "act":
                    # rec = 1/d2 in bf16 straight out of PSUM on ACT.
                    for t in range(TCH):
                        rec = ndp.tile([128, WBUF], bf16, tag="rec")
                        _act_reciprocal(nc, mybir, rec[:, :W], pds[t][:, :W])
                        recs.append(rec)
                else:
                    # f32 approx reciprocal on DVE, cast to bf16 on ACT.
                    for t in range(TCH):
                        rf = ndp.tile([128, WBUF], f32, tag="rf")
                        nc.vector.reciprocal_approx_fast(
                            out=rf[:, :W], in_=pds[t][:, :W]
                        )
                        rec = ndp.tile([128, WBUF], bf16, tag="rec")
                        nc.scalar.activation(
                            rec[:, :W], rf[:, :W], AF.Copy
                        )
                        recs.append(rec)
                for t in range(TCH):
                    top8 = small.tile([128, 8], bf16, tag="top8")
                    nc.vector.max(out=top8, in_=recs[t][:, :W])
                    tops.append(top8)
                sumw = small.tile([128, TMAX], f32, tag="sumw")
                wtss = []
                for t in range(TCH):
                    Wt_ = wp.tile([128, TMAX * 128], bf16, tag="W")
                    nc.vector.scalar_tensor_tensor(
                        out=Wt_[:, :W],
                        in0=recs[t][:, :W],
                        scalar=tops[t][:, 7:8],
                        in1=recs[t][:, :W],
                        op0=OP.is_ge,
                        op1=OP.mult,
                        accum_out=sumw[:, t : t + 1],
                    )
                    # W^T on the DMA xbar (16x128 tiles, bf16).  Rows of
                    # wts beyond the real source count are garbage; the
                    # numer matmul only reads [:cw].
                    wts = wtsp.tile([128, TMAX, 128], bf16, tag="wts")
                    eng = nc.sync if t % 2 == 0 else nc.scalar
                    eng.dma_start_transpose(
                        wts[:, :SCH, :], Wt_[:, : SCH * 128]
                    )
                    wtss.append(wts)
                rsw = small.tile([128, TMAX], f32, tag="rsw")
                nc.vector.reciprocal(rsw[:, :TCH], sumw[:, :TCH])
                ci0 = int(to[s])
                rswm = small.tile([128, TMAX], f32, tag="rswm")
                nc.vector.tensor_mul(
                    rswm[:, :TCH], rsw[:, :TCH], msk_t[:, ci0 : ci0 + TCH]
                )
                state[s] = (wtss, rswm)

            def emit_back(s):
                W, SCH, TCH = slot_shapes[s]
                wtss, rswm = state.pop(s)
                pis = []
                for t in range(TCH):
                    pi = pip_.tile([128, D], f32, tag="pi")
                    for k in range(SCH):
                        w0 = 128 * k
                        cw = min(W, w0 + 128) - w0
                        nc.tensor.matmul(
                            pi,
                            wtss[t][:cw, k],
                            f1a_t[:cw, int(fo[s]) + k],
                            start=(k == 0),
                            stop=(k == SCH - 1),
                        )
                    pis.append(pi)
                err = errp.tile([128, TMAX, D], bf16, tag="err")
                ci0 = int(to[s])
                for t in range(TCH):
                    if t == 1:
                        # balance: 1 of 3 chunks computes err on DVE in one
                        # fused op instead of ACT scaled-copy + Pool sub.
                        nc.vector.scalar_tensor_tensor(
                            out=err[:, t],
                            in0=pis[t],
                            scalar=rswm[:, t : t + 1],
                            in1=f2_t[:, ci0 + t],
                            op0=OP.mult,
                            op1=OP.subtract,
                        )
                    else:
                        tmp = errp.tile([128, D], bf16, tag="tmp")
                        nc.scalar.activation(
                            tmp, pis[t], AF.Copy, scale=rswm[:, t : t + 1]
                        )
                        nc.gpsimd.tensor_sub(
                            err[:, t], tmp, f2_t[:, ci0 + t]
                        )
                sq = errp.tile([128, TMAX, D], bf16, tag="sq")
                nc.vector.scalar_tensor_tensor(
                    out=sq[:, :TCH],
                    in0=err[:, :TCH],
                    scalar=1.0,
                    in1=err[:, :TCH],
                    op0=OP.mult,
                    op1=OP.mult,
                    accum_out=acc[:, s : s + 1],
                )

            for s in range(NSLOTS + LOOK):
                if s < NSLOTS:
                    emit_front(s)
                if s >= LOOK:
                    emit_back(s - LOOK)

            tot = constp.tile([128, 1], f32)
            nc.vector.reduce_sum(tot, acc, axis=mybir.AxisListType.X)
            nc.sync.dma_start(out_d[:, :], tot)

    nc.compile()
    return nc


def _hl(x):
    """fp16 hi/lo split: x ~= hi + lo with both parts exact in fp16."""
    hi = x.astype(np.float16)
    lo = (x - hi.astype(np.float32)).astype(np.float16)
    return hi, lo


def _prep(inputs):
    import ml_dtypes

    x1 = np.ascontiguousarray(np.asarray(inputs["x1"], dtype=np.float32))
    x2 = np.ascontiguousarray(np.asarray(inputs["x2"], dtype=np.float32))
    b1 = np.asarray(inputs["b1"]).astype(np.int64)
    b2 = np.asarray(inputs["b2"]).astype(np.int64)

    c1, f1 = x1[:, :3], x1[:, 3:]
    c2, f2 = x2[:, :3], x2[:, 3:]

    gs = np.arange(B + 1)
    e1 = np.searchsorted(b1, gs)
    e2 = np.searchsorted(b2, gs)
    n1 = np.diff(e1)
    n2 = np.diff(e2)
    assert n1.max() <= WBUF, f"source count {n1.max()} exceeds {WBUF}"
    assert n2.max() <= TMAX * 128, f"target count {n2.max()} exceeds {TMAX * 128}"
    assert n1.min() >= KNN, f"graph with fewer than {KNN} sources"

    tch = (n2 + 127) // 128
    # Slot assignment: graphs with more target chunks first (n1 desc within
    # the group); remaining graphs n1 ASC first within the slot that mixes
    # chunk counts, so the mixed slot stays narrow, then n1 desc.
    order = np.lexsort((-n1, -tch))
    tmax_cnt = int((tch == tch.max()).sum())
    fill = (-tmax_cnt) % NCORES
    if fill:
        rest = order[tmax_cnt:]
        rest = np.concatenate([rest[-fill:][::-1], rest[:-fill]])
        order = np.concatenate([order[:tmax_cnt], rest])
    slot_shapes = []
    for s in range(NSLOTS):
        gsl = order[s * NCORES : (s + 1) * NCORES]
        W = int(n1[gsl].max())
        slot_shapes.append((W, (W + 127) // 128, int(tch[gsl].max())))
    SCHs = [sh[1] for sh in slot_shapes]
    TCHs = [sh[2] for sh in slot_shapes]
    fo = np.concatenate([[0], np.cumsum(SCHs)])
    to = np.concatenate([[0], np.cumsum(TCHs)])
    FTOT = int(fo[-1])
    CTOT = int(to[-1])

    c1r = np.zeros((NCORES, KMM, NSLOTS, WBUF), np.float16)
    c2t = np.zeros((NCORES, KMM, NSLOTS, TMAX, 128), np.float16)
    f1a = np.zeros((NCORES, 128, FTOT, D), np.float32)
    f2p = np.zeros((NCORES, 128, CTOT, D), np.float32)
    msk = np.zeros((NCORES, 128, CTOT), np.float32)

    for rank, g in enumerate(order):
        s, core = divmod(rank, NCORES)
        W, SCH, TCH = slot_shapes[s]
        a, bb = e1[g], e1[g + 1]
        n = n1[g]
        cc = np.full((W, 3), BIGC, np.float32)
        cc[:n] = c1[a:bb]
        h1, l1 = _hl(cc)
        m2h1 = (-2.0 * h1.astype(np.float32)).astype(np.float16).T
        m2l1 = (-2.0 * l1.astype(np.float32)).astype(np.float16).T
        c1r[core, 0:3, s, :W] = m2h1
        c1r[core, 3:6, s, :W] = m2l1
        c1r[core, 6:9, s, :W] = m2h1
        nrm = np.einsum("ij,ij->i", cc, cc)
        nh, nl = _hl(nrm)
        c1r[core, 9, s, :W] = nh
        c1r[core, 10, s, :W] = nl
        c1r[core, 11:13, s, :W] = 1.0

        a2, bb2 = e2[g], e2[g + 1]
        m = n2[g]
        tcd = np.zeros((TCH * 128, 3), np.float32)
        tcd[:m] = c2[a2:bb2]
        h2, l2 = _hl(tcd)
        h2T = h2.T.reshape(3, TCH, 128)
        c2t[core, 0:3, s, :TCH] = h2T
        c2t[core, 3:6, s, :TCH] = h2T
        c2t[core, 6:9, s, :TCH] = l2.T.reshape(3, TCH, 128)
        c2t[core, 9:11, s, :TCH] = 1.0
        cn = np.einsum("ij,ij->i", tcd, tcd)
        ch, cl = _hl(cn)
        c2t[core, 11, s, :TCH] = ch.reshape(TCH, 128)
        c2t[core, 12, s, :TCH] = cl.reshape(TCH, 128)

        ff = np.zeros((SCH * 128, D), np.float32)
        ff[:n] = f1[a:bb]
        f1a[core, :, int(fo[s]) : int(fo[s]) + SCH] = ff.reshape(
            SCH, 128, D
        ).transpose(1, 0, 2)

        f2b = np.zeros((TCH * 128, D), np.float32)
        f2b[:m] = f2[a2:bb2]
        f2p[core, :, int(to[s]) : int(to[s]) + TCH] = f2b.reshape(
            TCH, 128, D
        ).transpose(1, 0, 2)
        msk[core, :, int(to[s]) : int(to[s]) + TCH] = (
            (np.arange(TCH * 128) < m).astype(np.float32).reshape(TCH, 128).T
        )

    in_maps = []
    for c in range(NCORES):
        in_maps.append(
            {
                "c1r": np.ascontiguousarray(c1r[c]),
                "c2t": np.ascontiguousarray(c2t[c]),
                "f1a": np.ascontiguousarray(f1a[c].astype(ml_dtypes.bfloat16)),
                "f2": np.ascontiguousarray(f2p[c].astype(ml_dtypes.bfloat16)),
                "msk": np.ascontiguousarray(msk[c]),
            }
        )
    return tuple(slot_shapes), in_maps


_NC_CACHE = {}


def run(inputs, trace=False):
    """Returns (mse_scalar_f32, exec_time_ns_or_None)."""
    from concourse.bass_utils import run_bass_kernel_spmd

    slot_shapes, in_maps = _prep(inputs)
    nc = _NC_CACHE.get(slot_shapes)
    if nc is None:
        nc = _NC_CACHE[slot_shapes] = _build_nc(slot_shapes)
    res = run_bass_kernel_spmd(
        nc, in_maps, core_ids=list(range(NCORES)), trace=trace
    )
    total = 0.0
    for r in res.results:
        total += np.asarray(r["out_sums"], dtype=np.float64).sum()
    mse = np.float32(total / (N * D))
    return mse, res.exec_time_ns


def kernel(**inputs):
    out, _ = run(inputs, trace=False)
    return out


# revision 42
# speedup vs baseline: 1.0481x; 1.0481x over previous
"""Bass/Trainium2 kernel for batched kNN-interpolate + MSE (nn_KnnMSE).

Reference computation:
  d2[i,j] = ||c2_i - c1_j||^2, masked to same-graph pairs (b1/b2 sorted),
  top-k=8 smallest per target row, w = 1/clip(d2, 1e-16),
  interp = sum(w * f1[idx]) / sum(w),  out = mean((interp - f2)^2).

v3 strategy:
  * b1/b2 sorted => block-diagonal over 64 graphs; one graph per core per
    "slot" (8 slots x 8 cores).  Slot shapes (source width W_s, source
    128-blocks SCH_s, target chunks TCH_s) specialized to the ACTUAL graph
    sizes at call time (graphs sorted by size, dealt into slots of 8; all
    cores run one shared program).
  * All inputs DMA'd into SBUF up front (~10 large DMAs, ~2MB resident).
  * Distance matmul folds in BOTH norm terms (KMM=13 fp16 hi/lo rows):
    PSUM holds d2 directly; no relu needed (d2 >= 2.7e-4 > 0 always).
  * bf16 selection pipeline at DVE 2x rate: negd2 = -d2 (one ACT scaled
    copy, PSUM->SBUF bf16); top8 = max8(negd2); the top-8 weight matrix
    comes out of ONE fused DVE op: W = (negd2 >= thresh) / negd2 (is_ge +
    divide) = -(1/d2) at the 8 nearest, 0 elsewhere, with sumw as the free
    accumulator output.  The minus signs cancel in interp = numer*rsw.
    This removes the full-width f32 reciprocal entirely.
  * W^T via the DMA xbar transpose (16x128 tiles, bf16) on the ~90% idle
    DMA engines - no PE transpose matmuls, no ACT PSUM->SBUF copy.
  * interp = ACT scaled copy of the numer PSUM (scale = mask/sumw rolled
    into one per-row scalar); err = pool subtract (bf16); sum of squares
    via one DVE fused multiply-accumulate per chunk.

Per 128-target chunk:
  PE   : psum = d2   (13-row fp16 hi/lo matmul, W_s wide)
  ACT  : nd = -d2    (scaled copy, bf16)
  DVE  : top8 = max8(nd); W,sumw = (nd >= top8[:,7]) / nd  [one fused op]
  DVE  : rsw = 1/sumw; rswm = rsw * mask
  DMA  : wts = W^T   (xbar transpose, SCH_s x [16,128] tiles)
  PE   : numer = wts^T @ f1  (bf16, PSUM-accum over SCH_s blocks)
  ACT  : interp = numer * rswm  (scaled copy, bf16)
  Pool : err = interp - f2      (bf16; f2/interp are 0 on padded rows)
  DVE  : acc[:, ci] += sum_d err*err  [fused mult-mult + accum]
  finally: per-partition totals -> DRAM; host sums 8x128 values / (N*D).

Self-contained: hardcodes problem constants; slot shapes come from the
actual b1/b2 at call time (host-side prep; program cached per shape set).
"""

import os as _os

import numpy as np

# Problem constants
N = 16384
D = 128
B = 64
KNN = 8
NCORES = 8
NSLOTS = B // NCORES     # 8 slots, one graph per core each
WBUF = 320               # SBUF buffer width for sources (max n1 <= 292)
TMAX = 3                 # max 128-row target chunks per graph
KMM = 13                 # dist contraction: 3x3 hi/lo cross + 2 n1 + 2 n2 rows
BIGC = 100.0             # padded-source coordinate => d2 ~ 3e4 >> real d2

PIPE = _os.environ.get("KNN_PIPE", "1") == "1"
RECIP = _os.environ.get("KNN_RECIP", "act")   # act | f32


def _act_reciprocal(nc, mybir, out, in_, accum_out=None):
    """ACT-engine table reciprocal (bass blocks AF.Reciprocal behind a
    precision ValueError; table accuracy ~1e-3 rel is plenty here)."""
    eng = nc.scalar
    AF = mybir.ActivationFunctionType
    ins = [
        eng.lower_ap(in_),
        mybir.ImmediateValue(dtype=mybir.dt.float32, value=0.0),  # bias
        mybir.ImmediateValue(dtype=mybir.dt.float32, value=1.0),  # scale
        mybir.ImmediateValue(dtype=mybir.dt.float32, value=0.0),  # alpha
    ]
    outs = [eng.lower_ap(out)]
    if accum_out is not None:
        outs.append(eng.lower_ap(accum_out))
    return eng.add_instruction(
        mybir.InstActivation(
            name=nc.get_next_instruction_name(),
            func=AF.Reciprocal,
            ins=ins,
            outs=outs,
        )
    )


def _build_nc(slot_shapes):
    import concourse.bacc as bacc
    import concourse.mybir as mybir
    import concourse.tile as tile

    f32 = mybir.dt.float32
    f16 = mybir.dt.float16
    bf16 = mybir.dt.bfloat16
    OP = mybir.AluOpType
    AF = mybir.ActivationFunctionType

    SCHs = [sh[1] for sh in slot_shapes]
    TCHs = [sh[2] for sh in slot_shapes]
    fo = np.concatenate([[0], np.cumsum(SCHs)])   # f1a block offsets
    to = np.concatenate([[0], np.cumsum(TCHs)])   # chunk offsets
    FTOT = int(fo[-1])
    CTOT = int(to[-1])

    nc = bacc.Bacc("TRN2", target_bir_lowering=False, debug=False)

    c1r_d = nc.dram_tensor("c1r", [KMM, NSLOTS, WBUF], f16, kind="ExternalInput")
    c2t_d = nc.dram_tensor("c2t", [KMM, NSLOTS, TMAX, 128], f16, kind="ExternalInput")
    f1a_d = nc.dram_tensor("f1a", [128, FTOT, D], bf16, kind="ExternalInput")
    f2_d = nc.dram_tensor("f2", [128, CTOT, D], bf16, kind="ExternalInput")
    msk_d = nc.dram_tensor("msk", [128, CTOT], f32, kind="ExternalInput")
    out_d = nc.dram_tensor("out_sums", [128, 1], f32, kind="ExternalOutput")

    LOOK = int(_os.environ.get("KNN_LOOK", "4")) if PIPE else 0
    NW = 3 * (LOOK + 1) + 3
    with tile.TileContext(nc) as tc:
        with (
            tc.tile_pool(name="constp", bufs=1) as constp,
            tc.tile_pool(name="ndp", bufs=8) as ndp,
            tc.tile_pool(name="wp", bufs=NW) as wp,
            tc.tile_pool(name="wtsp", bufs=NW) as wtsp,
            tc.tile_pool(name="errp", bufs=6) as errp,
            tc.tile_pool(name="small", bufs=2 * (LOOK + 2)) as small,
            tc.tile_pool(name="pdp", bufs=5, space="PSUM") as pdp,
            tc.tile_pool(name="pip", bufs=3, space="PSUM") as pip_,
        ):
            acc = constp.tile([128, NSLOTS], f32)
            nc.vector.memset(acc, 0.0)

            # Resident inputs, loaded once up front.
            c1r_t = constp.tile([KMM, NSLOTS, WBUF], f16)
            c2t_t = constp.tile([KMM, NSLOTS, TMAX, 128], f16)
            f1a_t = constp.tile([128, FTOT, D], bf16)
            f2_t = constp.tile([128, CTOT, D], bf16)
            msk_t = constp.tile([128, CTOT], f32)

            nc.sync.dma_start(c1r_t, c1r_d[:, :, :])
            nc.gpsimd.dma_start(c2t_t, c2t_d[:, :, :, :])
            nc.gpsimd.dma_start(msk_t, msk_d[:, :])
            # f1a/f2 are large; split across DMA queues, slot-bounds aligned.
            h1 = int(fo[NSLOTS // 2])
            nc.sync.dma_start(f1a_t[:, :h1], f1a_d[:, :h1])
            nc.sync.dma_start(f1a_t[:, h1:], f1a_d[:, h1:])
            qs = [int(to[i]) for i in (0, 2, 4, 6)] + [CTOT]
            for a, b in zip(qs[:-1], qs[1:]):
                nc.gpsimd.dma_start(f2_t[:, a:b], f2_d[:, a:b])

            state = {}

            def emit_front(s):
                W, SCH, TCH = slot_shapes[s]
                pds, recs, tops, rsws = [], [], [], []
                for t in range(TCH):
                    pd = pdp.tile([128, WBUF], f32, tag="pd")
                    nc.tensor.matmul(
                        pd[:, :W], c2t_t[:, s, t], c1r_t[:, s, :W],
                        start=True, stop=True,
                    )
                    pds.append(pd)
                if RECIP == "act":
                    # rec = 1/d2 in bf16 straight out of PSUM on ACT.
                    for t in range(TCH):
                        rec = ndp.tile([128, WBUF], bf16, tag="rec")
                        _act_reciprocal(nc, mybir, rec[:, :W], pds[t][:, :W])
                        recs.append(rec)
                else:
                    # f32 approx reciprocal on DVE, cast to bf16 on ACT.
                    for t in range(TCH):
                        rf = ndp.tile([128, WBUF], f32, tag="rf")
                        nc.vector.reciprocal_approx_fast(
                            out=rf[:, :W], in_=pds[t][:, :W]
                        )
                        rec = ndp.tile([128, WBUF], bf16, tag="rec")
                        nc.scalar.activation(
                            rec[:, :W], rf[:, :W], AF.Copy
                        )
                        recs.append(rec)
                for t in range(TCH):
                    top8 = small.tile([128, 8], bf16, tag="top8")
                    nc.vector.max(out=top8, in_=recs[t][:, :W])
                    tops.append(top8)
                sumw = small.tile([128, TMAX], f32, tag="sumw")
                wtss = []
                for t in range(TCH):
                    Wt_ = wp.tile([128, TMAX * 128], bf16, tag="W")
                    nc.vector.scalar_tensor_tensor(
                        out=Wt_[:, :W],
                        in0=recs[t][:, :W],
                        scalar=tops[t][:, 7:8],
                        in1=recs[t][:, :W],
                        op0=OP.is_ge,
                        op1=OP.mult,
                        accum_out=sumw[:, t : t + 1],
                    )
                    # W^T on the DMA xbar (16x128 tiles, bf16).  Rows of
                    # wts beyond the real source count are garbage; the
                    # numer matmul only reads [:cw].
                    wts = wtsp.tile([128, TMAX, 128], bf16, tag="wts")
                    eng = nc.sync if t % 2 == 0 else nc.scalar
                    eng.dma_start_transpose(
                        wts[:, :SCH, :], Wt_[:, : SCH * 128]
                    )
                    wtss.append(wts)
                rsw = small.tile([128, TMAX], f32, tag="rsw")
                nc.vector.reciprocal(rsw[:, :TCH], sumw[:, :TCH])
                ci0 = int(to[s])
                rswm = small.tile([128, TMAX], f32, tag="rswm")
                nc.vector.tensor_mul(
                    rswm[:, :TCH], rsw[:, :TCH], msk_t[:, ci0 : ci0 + TCH]
                )
                state[s] = (wtss, rswm)

            def emit_back(s):
                W, SCH, TCH = slot_shapes[s]
                wtss, rswm = state.pop(s)
                pis = []
                for t in range(TCH):
                    pi = pip_.tile([128, D], f32, tag="pi")
                    for k in range(SCH):
                        w0 = 128 * k
                        cw = min(W, w0 + 128) - w0
                        nc.tensor.matmul(
                            pi,
                            wtss[t][:cw, k],
                            f1a_t[:cw, int(fo[s]) + k],
                            start=(k == 0),
                            stop=(k == SCH - 1),
                        )
                    pis.append(pi)
                err = errp.tile([128, TMAX, D], bf16, tag="err")
                ci0 = int(to[s])
                for t in range(TCH):
                    if t == 1:
                        # balance: 1 of 3 chunks computes err on DVE in one
                        # fused op instead of ACT scaled-copy + Pool sub.
                        nc.vector.scalar_tensor_tensor(
                            out=err[:, t],
                            in0=pis[t],
                            scalar=rswm[:, t : t + 1],
                            in1=f2_t[:, ci0 + t],
                            op0=OP.mult,
                            op1=OP.subtract,
                        )
                    else:
                        tmp = errp.tile([128, D], bf16, tag="tmp")
                        nc.scalar.activation(
                            tmp, pis[t], AF.Copy, scale=rswm[:, t : t + 1]
                        )
                        nc.gpsimd.tensor_sub(
                            err[:, t], tmp, f2_t[:, ci0 + t]
                        )
                sq = errp.tile([128, TMAX, D], bf16, tag="sq")
                nc.vector.scalar_tensor_tensor(
                    out=sq[:, :TCH],
                    in0=err[:, :TCH],
                    scalar=1.0,
                    in1=err[:, :TCH],
                    op0=OP.mult,
                    op1=OP.mult,
                    accum_out=acc[:, s : s + 1],
                )

            for s in range(NSLOTS + LOOK):
                if s < NSLOTS:
                    emit_front(s)
                if s >= LOOK:
                    emit_back(s - LOOK)

            tot = constp.tile([128, 1], f32)
            nc.vector.reduce_sum(tot, acc, axis=mybir.AxisListType.X)
            nc.sync.dma_start(out_d[:, :], tot)

    nc.compile()
    return nc


def _hl(x):
    """fp16 hi/lo split: x ~= hi + lo with both parts exact in fp16."""
    hi = x.astype(np.float16)
    lo = (x - hi.astype(np.float32)).astype(np.float16)
    return hi, lo


def _prep(inputs):
    import ml_dtypes

    x1 = np.ascontiguousarray(np.asarray(inputs["x1"], dtype=np.float32))
    x2 = np.ascontiguousarray(np.asarray(inputs["x2"], dtype=np.float32))
    b1 = np.asarray(inputs["b1"]).astype(np.int64)
    b2 = np.asarray(inputs["b2"]).astype(np.int64)

    c1, f1 = x1[:, :3], x1[:, 3:]
    c2, f2 = x2[:, :3], x2[:, 3:]

    gs = np.arange(B + 1)
    e1 = np.searchsorted(b1, gs)
    e2 = np.searchsorted(b2, gs)
    n1 = np.diff(e1)
    n2 = np.diff(e2)
    assert n1.max() <= WBUF, f"source count {n1.max()} exceeds {WBUF}"
    assert n2.max() <= TMAX * 128, f"target count {n2.max()} exceeds {TMAX * 128}"
    assert n1.min() >= KNN, f"graph with fewer than {KNN} sources"

    tch = (n2 + 127) // 128
    # Slot assignment: graphs with more target chunks first (n1 desc within
    # the group); remaining graphs n1 ASC first within the slot that mixes
    # chunk counts, so the mixed slot stays narrow, then n1 desc.
    order = np.lexsort((-n1, -tch))
    tmax_cnt = int((tch == tch.max()).sum())
    fill = (-tmax_cnt) % NCORES
    if fill:
        rest = order[tmax_cnt:]
        rest = np.concatenate([rest[-fill:][::-1], rest[:-fill]])
        order = np.concatenate([order[:tmax_cnt], rest])
    slot_shapes = []
    for s in range(NSLOTS):
        gsl = order[s * NCORES : (s + 1) * NCORES]
        W = int(n1[gsl].max())
        slot_shapes.append((W, (W + 127) // 128, int(tch[gsl].max())))
    SCHs = [sh[1] for sh in slot_shapes]
    TCHs = [sh[2] for sh in slot_shapes]
    fo = np.concatenate([[0], np.cumsum(SCHs)])
    to = np.concatenate([[0], np.cumsum(TCHs)])
    FTOT = int(fo[-1])
    CTOT = int(to[-1])

    c1r = np.zeros((NCORES, KMM, NSLOTS, WBUF), np.float16)
    c2t = np.zeros((NCORES, KMM, NSLOTS, TMAX, 128), np.float16)
    f1a = np.zeros((NCORES, 128, FTOT, D), np.float32)
    f2p = np.zeros((NCORES, 128, CTOT, D), np.float32)
    msk = np.zeros((NCORES, 128, CTOT), np.float32)

    for rank, g in enumerate(order):
        s, core = divmod(rank, NCORES)
        W, SCH, TCH = slot_shapes[s]
        a, bb = e1[g], e1[g + 1]
        n = n1[g]
        cc = np.full((W, 3), BIGC, np.float32)
        cc[:n] = c1[a:bb]
        h1, l1 = _hl(cc)
        m2h1 = (-2.0 * h1.astype(np.float32)).astype(np.float16).T
        m2l1 = (-2.0 * l1.astype(np.float32)).astype(np.float16).T
        c1r[core, 0:3, s, :W] = m2h1
        c1r[core, 3:6, s, :W] = m2l1
        c1r[core, 6:9, s, :W] = m2h1
        nrm = np.einsum("ij,ij->i", cc, cc)
        nh, nl = _hl(nrm)
        c1r[core, 9, s, :W] = nh
        c1r[core, 10, s, :W] = nl
        c1r[core, 11:13, s, :W] = 1.0

        a2, bb2 = e2[g], e2[g + 1]
        m = n2[g]
        tcd = np.zeros((TCH * 128, 3), np.float32)
        tcd[:m] = c2[a2:bb2]
        h2, l2 = _hl(tcd)
        h2T = h2.T.reshape(3, TCH, 128)
        c2t[core, 0:3, s, :TCH] = h2T
        c2t[core, 3:6, s, :TCH] = h2T
        c2t[core, 6:9, s, :TCH] = l2.T.reshape(3, TCH, 128)
        c2t[core, 9:11, s, :TCH] = 1.0
        cn = np.einsum("ij,ij->i", tcd, tcd)
        ch, cl = _hl(cn)
        c2t[core, 11, s, :TCH] = ch.reshape(TCH, 128)
        c2t[core, 12, s, :TCH] = cl.reshape(TCH, 128)

        ff = np.zeros((SCH * 128, D), np.float32)
        ff[:n] = f1[a:bb]
        f1a[core, :, int(fo[s]) : int(fo[s]) + SCH] = ff.reshape(
            SCH, 128, D
        ).transpose(1, 0, 2)

        f2b = np.zeros((TCH * 128, D), np.float32)
        f2b[:m] = f2[a2:bb2]
        f2p[core, :, int(to[s]) : int(to[s]) + TCH] = f2b.reshape(
            TCH, 128, D
        ).transpose(1, 0, 2)
        msk[core, :, int(to[s]) : int(to[s]) + TCH] = (
            (np.arange(TCH * 128) < m).astype(np.float32).reshape(TCH, 128).T
        )

    in_maps = []
    for c in range(NCORES):
        in_maps.append(
            {
                "c1r": np.ascontiguousarray(c1r[c]),
                "c2t": np.ascontiguousarray(c2t[c]),
                "f1a": np.ascontiguousarray(f1a[c].astype(ml_dtypes.bfloat16)),
                "f2": np.ascontiguousarray(f2p[c].astype(ml_dtypes.bfloat16)),
                "msk": np.ascontiguousarray(msk[c]),
            }
        )
    return tuple(slot_shapes), in_maps


_NC_CACHE = {}


def run(inputs, trace=False):
    """Returns (mse_scalar_f32, exec_time_ns_or_None)."""
    from concourse.bass_utils import run_bass_kernel_spmd

    slot_shapes, in_maps = _prep(inputs)
    nc = _NC_CACHE.get(slot_shapes)
    if nc is None:
        nc = _NC_CACHE[slot_shapes] = _build_nc(slot_shapes)
    res = run_bass_kernel_spmd(
        nc, in_maps, core_ids=list(range(NCORES)), trace=trace
    )
    total = 0.0
    for r in res.results:
        total += np.asarray(r["out_sums"], dtype=np.float64).sum()
    mse = np.float32(total / (N * D))
    return mse, res.exec_time_ns


def kernel(**inputs):
    out, _ = run(inputs, trace=False)
    return out
